# revision 14
# baseline (speedup 1.0000x reference)
"""Multi-head attention (B=2, S=2048, D=1024, H=16, hd=64) on 8 trn2 cores.

Sharding: data parallel over batch (2) x tensor parallel over heads (4 groups
of 4 heads). Core c handles batch c//4, heads 4*(c%4)..4*(c%4)+3. Each core
projects Q/K/V for its head group (weights column-sharded), runs attention,
and computes a partial out-projection (Wo row-sharded); the host sums the 4
partials per batch and adds the output bias.

Per-core kernel layout notes:
- All inputs and weights are cast to bf16 host-side (halves DMA traffic; PE
  rate for bf16 equals f32r at N>=256, and fp8 quantization anywhere in the
  pipeline was measured to break the 2e-2 max-rel-err budget: concentrated
  softmax rows do not average the noise out).
- Q/K projections are computed transposed (QT/KT = [d', s], d' on partitions)
  so the scores matmul needs no on-chip transposes; V is computed in [s, d']
  layout and stored fp8e4 to serve as the PV DoubleRow stationary operand.
- Softmax skips the max-subtraction; a fixed bias of -2 inside the exp keeps
  probs within fp8e4 range (observed max z = 8.29 for this problem); the bias
  cancels in the softmax ratio. exp() output is written directly as fp8e4.
- Scores are computed transposed ([ks, qs]) in bf16 with the two heads of a
  pair packed into disjoint 64-row groups of the PE array (tile_position).
- PV and the out-projection run fp8e4 DoubleRow (2 contraction rows/cycle):
  probs tiles hold two k-blocks [128, 2, 1024]; ctx is stored fp8e4 (scaled
  x8 via the reciprocal) and contracted against Wo (x16) in one DoubleRow
  matmul per 512-wide output chunk; the final copy scales by 1/128.
- The normalizer broadcast runs on GpSimd (partition_broadcast); the
  normalize multiply is a DVE scalar_tensor_tensor writing fp8 ctx directly.
"""

import numpy as np
import ml_dtypes

import concourse.bass as bass
import concourse.tile as tile
from concourse import bacc, mybir
from concourse.bass_utils import run_bass_kernel_spmd

f32 = mybir.dt.float32
bf16 = mybir.dt.bfloat16
f16 = mybir.dt.float16
AFT = mybir.ActivationFunctionType
ALU = mybir.AluOpType

B, S, D = 2, 2048, 1024
H, HD = 16, 64
G = 4            # head groups (tensor-parallel degree)
HPG = H // G     # heads per group/core = 4
DH = HPG * HD    # 256 per-core projection width
N_CORES = 8
SCALE = 1.0 / np.sqrt(HD)  # 0.125

KC = D // 128    # 8 contraction chunks
SC = S // 128    # 16 s-chunks
JB = S // 512    # 4 qs blocks
MC = DH // 128   # 2 d'-chunks
HDP = HD + 1     # per-head V' width (64 V cols + ones column)


def _emit(nc, tc, xqt, xkt, xvt, wq, wk, wv, wo, o):
    with (
        tc.tile_pool(name="persist", bufs=1) as pp,
        tc.tile_pool(name="xt", bufs=16) as xt_pool,
        tc.tile_pool(name="psb", bufs=6) as psb,
        tc.tile_pool(name="outp", bufs=2) as outp_pool,
        tc.tile_pool(name="small", bufs=1) as small,
    ):
        wq_sb = pp.tile([128, KC, DH], bf16, name="wq_sb")
        wk_sb = pp.tile([128, KC, DH], bf16, name="wk_sb")
        wv_sb = pp.tile([128, KC, DH], bf16, name="wv_sb")
        wo_sb = pp.tile([128, MC, D], bf16, name="wo_sb")
        qt_sb = pp.tile([128, MC, S], bf16, name="qt_sb")
        kt_sb = pp.tile([128, MC, S], bf16, name="kt_sb")
        vp_sb = pp.tile([128, SC, HPG * HDP], f16, name="vp_sb")
        ctx_sb = pp.tile([128, MC, S], bf16, name="ctx_sb")
        ones1 = pp.tile([128, 1], f32, name="ones1")

        nc.vector.memset(ones1[:], 1.0)
        # ones columns of V' (col 64 of each head's 65-wide group)
        vp4 = vp_sb.rearrange("p i (h x) -> p i h x", h=HPG)
        nc.vector.tensor_copy(
            vp4[:, :, :, HD:HD + 1],
            ones1.unsqueeze(1).unsqueeze(1).broadcast_to([128, SC, HPG, 1]),
        )

        def load_w(w_dram, w_sb):
            nc.sync.dma_start(
                w_sb[:], w_dram.rearrange("p (kc n) -> p kc n", kc=KC)
            )

        def load_x(x_dram, label):
            tiles = {}
            for blk in range(2):
                for kc in range(KC):
                    t = xt_pool.tile(
                        [128, 1024], bf16, name=f"x{label}_{kc}_{blk}", tag="xt"
                    )
                    nc.sync.dma_start(
                        t[:],
                        x_dram[
                            128 * kc:128 * (kc + 1), 1024 * blk:1024 * (blk + 1)
                        ],
                    )
                    tiles[kc, blk] = t
            return tiles

        with tc.tile_pool(name="ps", bufs=1, space="PSUM") as ps:
            # DMA issue order follows first use: V weights + activations
            # stream first so the V projection starts as early as possible.
            load_w(wv, wv_sb)
            xv = load_x(xvt, "v")
            load_w(wk, wk_sb)
            xk = load_x(xkt, "k")
            load_w(wq, wq_sb)
            xq = load_x(xqt, "q")
            nc.sync.dma_start(
                wo_sb[:], wo.rearrange("p (mc n) -> p mc n", mc=MC)
            )

            def emit_v(i):
                blk, ii = divmod(i, 8)
                pv = ps.tile([128, DH], f32, name=f"pv_{i}", tag="ctxop",
                             bufs=4)
                for kc in range(KC):
                    nc.tensor.matmul(
                        pv[:],
                        xv[kc, blk][:, 128 * ii:128 * ii + 128],
                        wv_sb[:, kc, :],
                        start=(kc == 0),
                        stop=(kc == KC - 1),
                    )
                nc.vector.tensor_copy(
                    vp4[:, i, :, 0:HD],
                    pv.rearrange("p (h d) -> p h d", h=HPG),
                )

            def emit_proj(xt, w_sb, dst, label, jb, m):
                blk, jj = divmod(jb, 2)
                pt = ps.tile([128, 512], f32, name=f"p{label}_{jb}_{m}",
                             tag="ctxop", bufs=4)
                for kc in range(KC):
                    nc.tensor.matmul(
                        pt[:],
                        w_sb[:, kc, 128 * m:128 * m + 128],
                        xt[kc, blk][:, 512 * jj:512 * jj + 512],
                        start=(kc == 0),
                        stop=(kc == KC - 1),
                    )
                nc.vector.tensor_copy(dst[:, m, 512 * jb:512 * jb + 512], pt[:])

            op_chunks = []

            def make_op(sc, on_act=False):
                def emit():
                    ot = outp_pool.tile([128, 1024], bf16, name=f"ot_{sc}",
                                        tag="ot")
                    for eb in range(2):
                        po = ps.tile([128, 512], f32, name=f"po_{sc}_{eb}",
                                     tag="ctxop", bufs=4)
                        for mc in range(MC):
                            nc.tensor.matmul(
                                po[:],
                                ctx_sb[:, mc, 128 * sc:128 * sc + 128],
                                wo_sb[:, mc, 512 * eb:512 * eb + 512],
                                start=(mc == 0), stop=(mc == MC - 1),
                            )
                        dst = ot[:, 512 * eb:512 * eb + 512]
                        if on_act:
                            nc.scalar.copy(dst, po[:])
                        else:
                            nc.vector.tensor_copy(dst, po[:])
                    nc.sync.dma_start(o[128 * sc:128 * sc + 128, :], ot[:])
                return emit

            def make_pv(p, i, h0, h1, ctx0, ctx1):
                def emit():
                    nc.tensor.matmul(
                        ctx0[0:65, :],
                        vp_sb[:, i, HDP * h0:HDP * h0 + 65],
                        p[:, 0:512],
                        start=(i == 0), stop=(i == SC - 1),
                    )
                    nc.tensor.matmul(
                        ctx1[0:65, :],
                        vp_sb[:, i, HDP * h1:HDP * h1 + 65],
                        p[:, 512:1024],
                        start=(i == 0), stop=(i == SC - 1),
                    )
                return emit

            def emit_attn(jb, hp):
                q0 = 512 * jb
                h0, h1 = 2 * hp, 2 * hp + 1
                m = hp
                ctx0 = ps.tile([128, 512], f32, name=f"ctx0_{jb}_{hp}",
                               tag="ctxop", bufs=4)
                ctx1 = ps.tile([128, 512], f32, name=f"ctx1_{jb}_{hp}",
                               tag="ctxop", bufs=4)
                pend_pv = []
                for i in range(SC):
                    k0 = 128 * i
                    st = ps.tile([128, 1024], f32, name=f"st_{jb}_{hp}_{i}",
                                 tag="st", bufs=2)
                    nc.tensor.matmul(
                        st[:, 0:512],
                        kt_sb[0:64, m, k0:k0 + 128],
                        qt_sb[0:64, m, q0:q0 + 512],
                        start=True, stop=True, tile_position=(0, 0),
                    )
                    nc.tensor.matmul(
                        st[:, 512:1024],
                        kt_sb[64:128, m, k0:k0 + 128],
                        qt_sb[64:128, m, q0:q0 + 512],
                        start=True, stop=True, tile_position=(64, 0),
                    )
                    p = psb.tile([128, 1024], f16, name=f"p_{jb}_{hp}_{i}",
                                 tag="p")
                    nc.scalar.activation(p[:], st[:], AFT.Exp, scale=SCALE)
                    pend_pv.append(make_pv(p, i, h0, h1, ctx0, ctx1))
                    if i % 2 == 1:
                        while len(pend_pv) > 2:
                            pend_pv.pop(0)()
                    if hp == 0 and 1 <= i < 5 and op_chunks:
                        op_chunks.pop(0)()
                for f in pend_pv:
                    f()
                # normalize: ctx rows x (CTX_SCALE / denom row) -> fp8 ctx_sb
                for h, cps in ((h0, ctx0), (h1, ctx1)):
                    den = small.tile([1, 512], f32, name=f"den_{jb}_{h}",
                                     tag="den", bufs=2)
                    nc.vector.tensor_copy(den[:], cps[64:65, :])
                    rec = small.tile([1, 512], f32, name=f"rec_{jb}_{h}",
                                     tag="rec", bufs=2)
                    nc.vector.reciprocal_approx_fast(out=rec[:], in_=den[:])
                    bcs = small.tile([64, 512], f32, name=f"bcs_{jb}_{h}",
                                     tag="bcs", bufs=2)
                    nc.gpsimd.partition_broadcast(bcs[:], rec[:])
                    rr = 64 * (h % 2)
                    nc.vector.scalar_tensor_tensor(
                        ctx_sb[rr:rr + 64, m, q0:q0 + 512],
                        cps[0:64, :],
                        1.0,
                        bcs[:],
                        ALU.mult,
                        ALU.mult,
                    )

            # emission schedule: V first (xv streams first), KT, then QT
            # block by block with attention pairs as soon as each QT block
            # lands; remaining projections fill attention bubbles.
            for i in range(SC):
                emit_v(i)
            for m in range(MC):
                for jb in range(JB):
                    emit_proj(xk, wk_sb, kt_sb, "k", jb, m)
            for jb in range(JB):
                emit_proj(xq, wq_sb, qt_sb, "q", jb, 0)
                emit_proj(xq, wq_sb, qt_sb, "q", jb, 1)
                emit_attn(jb, 0)
                emit_attn(jb, 1)
                assert not op_chunks
                op_chunks = [make_op(sc) for sc in range(4 * jb, 4 * jb + 4)]
            for f in op_chunks:
                f()


_CACHE = {}


def _get_nc():
    if "nc" not in _CACHE:
        nc = bacc.Bacc("TRN2", target_bir_lowering=False, debug=False,
                       num_devices=N_CORES)
        xqt = nc.dram_tensor("xqt", [D, S], bf16, kind="ExternalInput").ap()
        xkt = nc.dram_tensor("xkt", [D, S], bf16, kind="ExternalInput").ap()
        xvt = nc.dram_tensor("xvt", [D, S], bf16, kind="ExternalInput").ap()
        wq = nc.dram_tensor("wq", [128, KC * DH], bf16,
                            kind="ExternalInput").ap()
        wk = nc.dram_tensor("wk", [128, KC * DH], bf16,
                            kind="ExternalInput").ap()
        wv = nc.dram_tensor("wv", [128, KC * DH], bf16,
                            kind="ExternalInput").ap()
        wo = nc.dram_tensor("wo", [128, MC * D], bf16,
                            kind="ExternalInput").ap()
        o = nc.dram_tensor("o", [S, D], bf16, kind="ExternalOutput").ap()
        with tile.TileContext(nc) as tc:
            _emit(nc, tc, xqt, xkt, xvt, wq, wk, wv, wo, o)
        nc.compile()
        _CACHE["nc"] = nc
    return _CACHE["nc"]


def kernel(query, key, value, Wq, bq, Wk, bk, Wv, bv, Wo, bo, **run_kwargs):
    query = np.asarray(query, dtype=np.float32)
    key = np.asarray(key, dtype=np.float32)
    value = np.asarray(value, dtype=np.float32)
    Wq = np.asarray(Wq, dtype=np.float32)
    Wk = np.asarray(Wk, dtype=np.float32)
    Wv = np.asarray(Wv, dtype=np.float32)
    Wo = np.asarray(Wo, dtype=np.float32).astype(ml_dtypes.bfloat16)

    # bq/bk/bv are zero for this module (asserted); bo is applied host-side.
    for b_arr in (bq, bk, bv):
        assert not np.any(np.asarray(b_arr)), "nonzero qkv bias unsupported"

    def wlayout(w, chunks):
        # [(c p), n] -> [p, (c n)] so the weight DMA is contiguous rows
        cp, n = w.shape
        return np.ascontiguousarray(
            w.reshape(chunks, 128, n).transpose(1, 0, 2).reshape(128, -1))

    xt = {}
    for bi in range(B):
        xt["q", bi] = np.ascontiguousarray(query[bi].T).astype(
            ml_dtypes.bfloat16)
        xt["k", bi] = np.ascontiguousarray(key[bi].T).astype(
            ml_dtypes.bfloat16)
        xt["v", bi] = np.ascontiguousarray(value[bi].T).astype(
            ml_dtypes.bfloat16)

    in_maps = []
    for c in range(N_CORES):
        bi, g = divmod(c, G)
        cs = slice(DH * g, DH * (g + 1))
        in_maps.append({
            "xqt": xt["q", bi],
            "xkt": xt["k", bi],
            "xvt": xt["v", bi],
            "wq": wlayout(Wq[:, cs], KC).astype(ml_dtypes.bfloat16),
            "wk": wlayout(Wk[:, cs], KC).astype(ml_dtypes.bfloat16),
            "wv": wlayout(Wv[:, cs], KC).astype(ml_dtypes.bfloat16),
            "wo": wlayout(np.asarray(Wo[cs, :]), MC),
        })

    nc = _get_nc()
    res = run_bass_kernel_spmd(nc, in_maps, core_ids=list(range(N_CORES)),
                               **run_kwargs)

    out = np.empty((B, S, D), dtype=np.float32)
    for bi in range(B):
        acc = res.results[4 * bi]["o"].astype(np.float32)
        for g in range(1, G):
            acc = acc + res.results[4 * bi + g]["o"]
        out[bi] = acc
    out += np.asarray(bo, dtype=np.float32)[None, None, :]

    if run_kwargs:
        kernel.last_results = res
    return out
